# revision 34
# baseline (speedup 1.0000x reference)
"""Trainium2 Bass kernel for nn_Attention_57672820850902 (fp8 DoubleRow).

Channel-attention block (XCA-style):
  kv = dwconv3x3(conv1x1(x)); k, v = split(kv)
  q  = conv3x3_full(conv1x1(y))   [the two q convs are FUSED host-side]
  q, k l2-normalized per channel row; attn = softmax(q @ k^T * temp) per head
  out = x - conv1x1(attn @ v)

Sharding: 8 cores = 4 batches x 2 spatial halves (64 rows + 1-row halo).
Cross-core traffic: one pairwise AllReduce (149KB) carrying the per-head
logits (contracted over local spatial) + per-channel sums of squares.

Perf structure (352us baseline -> ~260us measured, best 258us):
- Heavy convs are fp8e4 DoubleRow matmuls: rhs [96, 2, N] with the 192
  channels split across the two K-streams (stream step = SROW, %16 == 0
  -- small or overlapping stream steps CRASH the hardware). One DR pass
  per tap covers all 192 input channels; DR passes measure 216ns at
  N=512, same as bf16, i.e. 2x FLOPs.
- Convs run FLAT over a shared-pad grid: rows 129 wide (right-pad of row
  r == left-pad of row r+1), 66 rows, grid at offset LEAD=130 so the
  tap window c0 + ky*129 + kx is always in range. Border outputs are
  junk; k-side borders are zeroed (their zeros also kill the q-side
  junk in the QK^T products); valid-window consumers (phase 8, ssq)
  read 4/8-row strided views instead.
- q 1x1+3x3 fused into one 3x3 conv on y (host-side weight composition,
  x8); kv weights x16; l2norm makes q/k scale-free. proj carries -16x
  and the residual STT multiplies the psum by 1/256. Output fp16.
- Depthwise 3x3: k-lower on VectorE (STT taps); [v-upper;k-upper] as
  bf16 diag-matmuls interleaved into phase 4 on the PE; v-lower as PE
  diag-matmuls dep-forced AFTER QK^T so they fill the AllReduce window.
- QK^T runs fp8 DoubleRow over spatial-block pairs (68 passes of ~170ns
  instead of 136 bf16 passes): q/k are transposed bf16 in 4-chunk
  groups (xbar transposes, sync queue only -- concurrent xbar
  transposes corrupt data) into staging tiles, then cast to fp8 on
  Vector/Scalar (NOT GpSimd -- its serial cast pacing starves QK^T).
- Scheduler discipline learned the hard way: the tile scheduler
  reorders by deps and semaphore waits get coarsened per engine FIFO,
  and a missed event costs a 10.4us poll. Keep the QK->AllReduce chain
  off the vector queue (v-diag evacs own it): logits extraction and the
  ssq/ssk reductions run as Scalar activation-accum ops writing straight
  into the 2-DMA collective payload (sequentially-armed multi-DMA waits
  each miss their completion event and poll); the EXP/SQRT ACT tables
  are preloaded off the critical path; proj lags attn@v by 4 chunks so
  the tail phases pipeline instead of ping-ponging across engines.
"""

import numpy as np
import ml_dtypes

B, C, H, W, HEADS = 4, 192, 128, 128, 6
HC = C // HEADS                      # 32 channels per head
HP = H // 2                          # 64 rows per core
PW = W + 1                           # 129: shared-pad row width
PH = HP + 2                          # 66 rows
SGRID = PH * PW                      # 8514
SROW = 8976                          # stream stride (>= 8964, %16 == 0)
LEAD = 130                           # grid offset inside x_dr/y_dr/v1a (flat-conv lead)
NCH = 17                             # 17 chunks of 512 cover 8704 >= SGRID
S_IN = HP * W                        # 8192 valid positions
NCC = S_IN // 512                    # 16 compact chunks
NCORES = 8
CA, CB = 128, 64

bf16 = ml_dtypes.bfloat16
f8 = ml_dtypes.float8_e4m3

SK = 16.0                            # kv_w scale (k and v paths)
SQ = 8.0                             # fused q weight scale (q must fit fp8 after cast)
INV_OUT = 1.0 / 256.0                # undo 16(v) * 16(proj) at the residual STT

_cache = {}


def _build():
    import concourse.bass as bass
    import concourse.mybir as mybir
    import concourse.tile as tile
    from concourse import bacc

    dt = mybir.dt
    Alu = mybir.AluOpType
    Act = mybir.ActivationFunctionType
    DR = mybir.MatmulPerfMode.DoubleRow

    nc = bacc.Bacc("TRN2", target_bir_lowering=False, debug=False,
                   num_devices=NCORES)

    x_dr_t = nc.dram_tensor("x_dr", [96, 2 * SROW], dt.float8e4, kind="ExternalInput")
    y_dr_t = nc.dram_tensor("y_dr", [96, 2 * SROW], dt.float8e4, kind="ExternalInput")
    xr_t = nc.dram_tensor("xr", [C, S_IN], dt.float16, kind="ExternalInput")
    kvw_t = nc.dram_tensor("kvw_dr", [96, 2 * 384], dt.float8e4, kind="ExternalInput")
    qwf_t = nc.dram_tensor("qwf_dr", [96, 9 * 2 * C], dt.float8e4, kind="ExternalInput")
    dwk_t = nc.dram_tensor("dw_k", [CA, 9], dt.float32, kind="ExternalInput")
    dwdm_t = nc.dram_tensor("dw_diag_m", [9, CA, CA], dt.bfloat16, kind="ExternalInput")
    dwdiag_t = nc.dram_tensor("dw_diag", [9, CA, CA], dt.bfloat16, kind="ExternalInput")
    projw_t = nc.dram_tensor("proj_dr", [CA, 2 * C], dt.float8e4, kind="ExternalInput")
    temp_t = nc.dram_tensor("temp", [HC, HEADS], dt.float32, kind="ExternalInput")
    out_t = nc.dram_tensor("out", [C, S_IN], dt.float16, kind="ExternalOutput")

    taps = [(ky, kx) for ky in range(3) for kx in range(3)]

    def gview(t, row0, col0, nrows=64, ncols=W):
        """[p, nrows, ncols] strided view of a flat grid tile at (row0, col0)."""
        off = row0 * PW + col0
        v = t[:, off:off + nrows * PW]
        return v.rearrange("p (h w) -> p h w", w=PW)[:, :, 0:ncols]

    with tile.TileContext(nc) as tc:
        with tc.tile_pool(name="w", bufs=1) as wp, \
             tc.tile_pool(name="big", bufs=1) as bigp, \
             tc.tile_pool(name="kt", bufs=18) as ktp, \
             tc.tile_pool(name="io2", bufs=4) as iop2, \
             tc.tile_pool(name="qts", bufs=2) as qtsp, \
             tc.tile_pool(name="ev", bufs=2) as evp, \
             tc.tile_pool(name="small", bufs=1) as smp, \
             tc.tile_pool(name="ps", bufs=6, space="PSUM") as psp, \
             tc.tile_pool(name="psattn", bufs=1, space="PSUM") as psattn, \
             tc.tile_pool(name="dram", bufs=1, space="DRAM") as dramp:

            # ---------- weights ----------
            kvw = wp.tile([96, 2, 384], dt.float8e4)
            nc.scalar.dma_start(kvw[:].rearrange("p a b -> p (a b)"), kvw_t.ap())
            qwf = wp.tile([96, 9, 2, C], dt.float8e4)
            nc.scalar.dma_start(qwf[:].rearrange("p t a b -> p (t a b)"), qwf_t.ap())
            dwk = wp.tile([CA, 9], dt.float32)
            nc.scalar.dma_start(dwk[:], dwk_t.ap())
            dwdm = wp.tile([CA, 9, CA], dt.bfloat16)
            nc.scalar.dma_start(dwdm[:], dwdm_t.ap().rearrange("t k m -> k t m"))
            dwdiag = wp.tile([CA, 9, CA], dt.bfloat16)
            nc.scalar.dma_start(dwdiag[:], dwdiag_t.ap().rearrange("t k m -> k t m"))
            projw = wp.tile([CA, 2, C], dt.float8e4)
            nc.scalar.dma_start(projw[:].rearrange("p a b -> p (a b)"), projw_t.ap())
            temp_s = wp.tile([HC, HEADS], dt.float32)
            nc.scalar.dma_start(temp_s[:], temp_t.ap())

            # ---------- persistent intermediates (tags manage slot reuse) ----------
            x_dr = bigp.tile([96, 2, SROW], dt.float8e4, tag="xdr")
            y_dr = bigp.tile([96, 2, SROW], dt.float8e4, tag="ydr")
            k1a = bigp.tile([CA, 8704], dt.bfloat16, tag="k1a")
            kv1b = bigp.tile([CA, SROW], dt.float8e4, tag="kv1b")
            v1a = bigp.tile([CA, SROW], dt.float8e4, tag="v1a")
            ka = bigp.tile([CA, 8704], dt.bfloat16, tag="ka")
            kvb = bigp.tile([CA, 8704], dt.bfloat16, tag="kvb")
            qa = bigp.tile([CA, 8704], dt.bfloat16, tag="xdr")    # after x_dr dies
            qb = bigp.tile([CB, 8704], dt.bfloat16, tag="qb")
            qt_full = bigp.tile([CA, 4 * NCH, C], dt.bfloat16, tag="qt")
            va = bigp.tile([CA, 8704], dt.bfloat16, tag="k1a")    # after k dw
            oh_dr = bigp.tile([CA, 2 * S_IN], dt.float8e4, tag="ydr")  # after y dies
            xr_a = bigp.tile([CA, S_IN], dt.float16, tag="ka")    # after kt transposes
            xr_b = bigp.tile([CB, S_IN], dt.float16, tag="qb")    # after q ssq/qt

            for tt in (v1a, kv1b):
                nc.vector.memset(tt[:, 0:LEAD], 0.0)
                nc.vector.memset(tt[:, LEAD + 8704:SROW], 0.0)

            attn_pa = psattn.tile([CA, C], dt.float32, tag="attnA")
            attn_pb = psattn.tile([CB, C], dt.float32, tag="attnB")

            # ---------- input loads: x in 4 slices (ph1 starts early), then y ----------
            QW = SROW // 4
            x_src = x_dr_t.ap().rearrange("p (a b) -> p a b", b=SROW)
            y_src = y_dr_t.ap().rearrange("p (a b) -> p a b", b=SROW)
            for j in range(4):
                nc.sync.dma_start(x_dr[:, :, QW * j:QW * (j + 1)],
                                  x_src[:, :, QW * j:QW * (j + 1)])
            for j in range(4):
                nc.scalar.dma_start(y_dr[:, :, QW * j:QW * (j + 1)],
                                    y_src[:, :, QW * j:QW * (j + 1)])

            # ---------- phase 1: kv1 = kv_w @ x over the padded grid ----------
            #   p0 = k[0:128]; p1 = [v 128:192 ; k 128:192]; p2 = v[0:128]
            for i in range(NCH):
                c0 = 512 * i
                p0 = psp.tile([CA, 512], dt.float32, tag="mm")
                p1 = psp.tile([CA, 512], dt.float32, tag="mm")
                p2 = psp.tile([CA, 512], dt.float32, tag="mm")
                rhs = x_dr[:, :, LEAD + c0:LEAD + c0 + 512]
                nc.tensor.matmul(p0[:], kvw[:, :, 0:128], rhs, start=True, stop=True,
                                 perf_mode=DR)
                nc.tensor.matmul(p1[:], kvw[:, :, 128:256], rhs, start=True, stop=True,
                                 perf_mode=DR)
                nc.tensor.matmul(p2[:], kvw[:, :, 256:384], rhs, start=True, stop=True,
                                 perf_mode=DR)
                nc.vector.tensor_copy(k1a[:, c0:c0 + 512], p0[:])
                nc.vector.tensor_copy(kv1b[:, LEAD + c0:LEAD + c0 + 512], p1[:])
                nc.scalar.copy(v1a[:, LEAD + c0:LEAD + c0 + 512], p2[:])

            # ---------- phase 2: depthwise 3x3 (VectorE) ----------
            def dw(dst, src, wsc):
                d = gview(dst[:], 1, 1)
                for t, (ky, kx) in enumerate(taps):
                    s = gview(src[:], ky, kx)
                    if t == 0:
                        nc.vector.tensor_scalar(d, s, wsc[:, 0:1], None, Alu.mult)
                    else:
                        nc.vector.scalar_tensor_tensor(d, s, wsc[:, t:t + 1], d,
                                                       Alu.mult, Alu.add)

            dw(ka, k1a, dwk)
            dw(kvb, kv1b, dwm)

            # border zeroing for the transpose/ssk consumers
            def zero_borders(t, plo, phi):
                nc.vector.memset(t[plo:phi, 0:PW + 1], 0.0)                  # row 0 + (1,0)
                cols = t[plo:phi, 0:66 * PW].rearrange("p (h w) -> p h w", w=PW)
                nc.vector.memset(cols[:, :, 0:1], 0.0)                       # left pads
                nc.vector.memset(t[plo:phi, 65 * PW:8704], 0.0)              # row 65 + tail

            zero_borders(ka, 0, CA)

            # ssk: sum of squares of k rows (runs early, overlapped with phase 4)
            ssk_a = smp.tile([CA, 9], dt.float32)
            ssk_u = smp.tile([CA, 9], dt.float32)   # rows 64:128 active
            for (srct, plo, phi, dst) in ((ka, 0, CA, ssk_a), (kvb, CB, CA, ssk_u)):
                for i in range(9):
                    c0, c1 = 1024 * i, min(1024 * (i + 1), 8704)
                    sq = evp.tile([CA, 1024], dt.bfloat16, tag="sqscr")
                    nc.scalar.activation(sq[plo:phi, 0:c1 - c0], srct[plo:phi, c0:c1],
                                         Act.Square, accum_out=dst[plo:phi, i:i + 1])

            # ---------- phase 4: fused q 3x3 conv + evac + border zero + transpose ----------
            ssq_a = smp.tile([CA, NCH], dt.float32)
            ssq_b = smp.tile([CB, NCH], dt.float32)
            for i in range(NCH):
                c0 = 512 * i
                pqa = psp.tile([CA, 512], dt.float32, tag="mm")
                pqb = psp.tile([CB, 512], dt.float32, tag="mm")
                for t, (ky, kx) in enumerate(taps):
                    rhs = y_dr[:, :, c0 + ky * PW + kx:c0 + ky * PW + kx + 512]
                    nc.tensor.matmul(pqa[:], qwf[:, t, :, 0:CA], rhs,
                                     start=(t == 0), stop=(t == 8), perf_mode=DR)
                for t, (ky, kx) in enumerate(taps):
                    rhs = y_dr[:, :, c0 + ky * PW + kx:c0 + ky * PW + kx + 512]
                    nc.tensor.matmul(pqb[:], qwf[:, t, :, CA:C], rhs,
                                     start=(t == 0), stop=(t == 8), perf_mode=DR)
                nc.vector.tensor_copy(qa[:, c0:c0 + 512], pqa[:])
                nc.scalar.copy(qb[:, c0:c0 + 512], pqb[:])
                # zero border positions inside this chunk (left pads; chunk 0 adds
                # row 0, chunk 16 adds row 65 + tail)
                for (tt, plo, phi) in ((qa, 0, CA), (qb, 0, CB)):
                    if i == 0:
                        nc.vector.memset(tt[plo:phi, 0:PW + 1], 0.0)
                    if i == NCH - 1:
                        nc.vector.memset(tt[plo:phi, 65 * PW:8704], 0.0)
                    # left pads (row r starts at r*PW) for grid rows 1..64 in chunk
                    r_lo = max(1, (c0 + PW - 1) // PW)
                    r_hi = min(64, (c0 + 511) // PW)
                    if r_lo <= r_hi:
                        bv = tt[plo:phi, r_lo * PW:(r_hi + 1) * PW]
                        nc.vector.memset(
                            bv.rearrange("p (h w) -> p h w", w=PW)[:, :, 0:1], 0.0)
                nc.sync.dma_start_transpose(qt_full[:, 4 * i:4 * (i + 1), 0:CA],
                                            qa[:, c0:c0 + 512])
                nc.sync.dma_start_transpose(qt_full[:, 4 * i:4 * (i + 1), CA:C],
                                            qb[:, c0:c0 + 512])
                sq = evp.tile([CA, 512], dt.bfloat16, tag="sqscr2")
                nc.scalar.activation(sq[:], qa[:, c0:c0 + 512], Act.Square,
                                     accum_out=ssq_a[:, i:i + 1])
                nc.scalar.activation(sq[0:CB], qb[:, c0:c0 + 512], Act.Square,
                                     accum_out=ssq_b[:, i:i + 1])

            # preload the scalar-engine EXP table so the softmax doesn't pay
            # the ACT_TABLE_LOAD on the critical path
            exp_warm = smp.tile([1, 8], dt.float32)
            nc.scalar.activation(exp_warm[:], ssk_a[0:1, 0:8], Act.Sqrt)

            # ---------- phase 4b: QK^T with just-in-time k transposes ----------
            for i in range(NCH):
                kt4 = kts[i]
                for j in range(4):
                    nc.tensor.matmul(attn_pa[:], qt_full[:, 4 * i + j, 0:CA],
                                     kt4[:, j, :],
                                     start=(i == 0 and j == 0),
                                     stop=(i == NCH - 1 and j == 3),
                                     skip_group_check=True)
                    qk_last = nc.tensor.matmul(attn_pb[:], qt_full[:, 4 * i + j, CA:C],
                                               kt4[:, j, :],
                                               start=(i == 0 and j == 0),
                                               stop=(i == NCH - 1 and j == 3),
                                               skip_group_check=True)

            # ---------- phase 4c: v-lower depthwise via PE diag (fills the gap;
            # explicitly ordered after QK^T so the scheduler cannot hoist it) ----------
            from concourse.tile_rust import add_dep_helper
            first_va = None
            for i in range(NCH):
                c0 = 512 * i
                pd = psp.tile([CA, 512], dt.float32, tag="mm")
                for t, (ky, kx) in enumerate(taps):
                    rhs = v1a[:, c0 + ky * PW + kx:c0 + ky * PW + kx + 512]
                    mi = nc.tensor.matmul(pd[:], dwdiag[:, t, :], rhs,
                                          start=(t == 0), stop=(t == 8))
                    if first_va is None:
                        first_va = mi
                nc.scalar.copy(va[:, c0:c0 + 512], pd[:])
            add_dep_helper(first_va.ins, qk_last.ins, sync=False,
                           reason="v-diag after QK^T so it fills the AllReduce window")

            # ---------- phase 6: pack + pairwise all-reduce ----------
            attn_sa = smp.tile([CA, C], dt.float32)
            attn_sb = smp.tile([CB, C], dt.float32)
            nc.vector.tensor_copy(attn_sa[:], attn_pa[:])
            nc.scalar.copy(attn_sb[:], attn_pb[:])
            cin = dramp.tile([34, C], dt.float32)
            cout = dramp.tile([34, C], dt.float32)
            for h in range(HEADS):
                src = attn_sa if h < 4 else attn_sb
                r = HC * (h % 4)
                nc.scalar.dma_start(cin[0:HC, HC * h:HC * (h + 1)],
                                    src[r:r + HC, HC * h:HC * (h + 1)])
            nc.scalar.dma_start(cin[32:33, 0:CA].rearrange("o c -> c o"), ssq[:, 0:1])
            nc.scalar.dma_start(cin[32:33, CA:C].rearrange("o c -> c o"), ssq[0:CB, 1:2])
            nc.scalar.dma_start(cin[33:34, 0:CA].rearrange("o c -> c o"), ssk[:, 0:1])
            nc.scalar.dma_start(cin[33:34, CA:C].rearrange("o c -> c o"), ssk[CB:CA, 1:2])
            nc.sync.dma_start(xr_a[:], xr_t.ap()[0:CA])
            nc.sync.dma_start(xr_b[:], xr_t.ap()[CA:C])
            nc.gpsimd.collective_compute(
                "AllReduce", Alu.add,
                replica_groups=[[0, 1], [2, 3], [4, 5], [6, 7]],
                ins=[cin[:].opt()], outs=[cout[:].opt()])

            # ---------- phase 7: softmax ----------
            attn_f = smp.tile([HC, HEADS, HC], dt.float32)
            nc.scalar.dma_start(attn_f[:], cout[0:HC, :].rearrange("p (h c) -> p h c", h=HEADS))
            fq = smp.tile([HC, HEADS], dt.float32)
            fk = smp.tile([1, C], dt.float32)
            nc.scalar.dma_start(fq[:], cout[32:33, :].rearrange("o (h c) -> c (o h)", h=HEADS))
            nc.scalar.dma_start(fk[:], cout[33:34, :])
            nc.vector.reciprocal(fq[:], fq[:])
            nc.vector.reciprocal(fk[:], fk[:])
            nc.scalar.activation(fq[:], fq[:], Act.Sqrt)
            nc.scalar.activation(fk[:], fk[:], Act.Sqrt)
            # preload the EXP table while the vector mults run
            nc.scalar.activation(exp_warm[:], exp_warm[:], Act.Exp)
            nc.vector.tensor_tensor(fq[:], fq[:], temp_s[:], Alu.mult)
            fk32 = smp.tile([HC, C], dt.float32)
            nc.gpsimd.partition_broadcast(fk32[:], fk[:])
            nc.vector.tensor_tensor(attn_f[:], attn_f[:],
                                    fq[:, :, None].to_broadcast((HC, HEADS, HC)), Alu.mult)
            nc.vector.tensor_tensor(attn_f[:], attn_f[:],
                                    fk32[:].rearrange("p (h c) -> p h c", h=HEADS), Alu.mult)
            ex = smp.tile([HC, HEADS, HC], dt.float32)
            nc.scalar.activation(ex[:], attn_f[:], Act.Exp)
            sm = smp.tile([HC, HEADS], dt.float32)
            nc.vector.tensor_reduce(sm[:], ex[:], mybir.AxisListType.X, Alu.add)
            nc.vector.reciprocal(sm[:], sm[:])
            nc.vector.tensor_tensor(ex[:], ex[:],
                                    sm[:, :, None].to_broadcast((HC, HEADS, HC)), Alu.mult)
            attn_bf = smp.tile([HC, HEADS, HC], dt.bfloat16)
            nc.vector.tensor_copy(attn_bf[:], ex[:])
            attn_T = smp.tile([HC, HEADS, HC], dt.bfloat16)
            nc.vector.transpose(attn_T[:].rearrange("p h c -> p (h c)"),
                                attn_bf[:].rearrange("p h c -> p (h c)"))
            bd_a = smp.tile([CA, CA], dt.bfloat16)
            bd_b = smp.tile([CA, CB], dt.bfloat16)   # rows 64:128 zero
            nc.vector.memset(bd_a[:], 0.0)
            nc.vector.memset(bd_b[:], 0.0)
            for h in range(HEADS):
                if h < 4:
                    nc.sync.dma_start(bd_a[HC * h:HC * (h + 1), HC * h:HC * (h + 1)],
                                      attn_T[:, h, :])
                else:
                    j = h - 4
                    nc.sync.dma_start(bd_b[HC * j:HC * (j + 1), HC * j:HC * (j + 1)],
                                      attn_T[:, h, :])

            # zero the junk rows of the odd (b) blocks of oh_dr once (y_dr slot is
            # dead by now); proj stream1 rows 64:128 are zero so these only need
            # to be finite
            nc.vector.memset(oh_dr[CB:CA, :].rearrange("p (n a b) -> p n a b",
                                                       a=2, b=512)[:, :, 1, :], 0.0)

            # ---------- phase 8: out_heads = attn @ v (compact grid) ----------
            for i in range(NCC):
                r0 = 4 * i
                pva = psp.tile([CA, 512], dt.float32, tag="mm")
                pvb = psp.tile([CB, 512], dt.float32, tag="mm")
                nc.tensor.matmul(pva[:], bd_a[:],
                                 gview(va[:], 1 + r0, 1, nrows=4), start=True, stop=True)
                # rhs rows 64:128 hold dw'd k-upper; bd_b zero rows cancel them
                nc.tensor.matmul(pvb[:], bd_b[:],
                                 gview(kvb[:], 1 + r0, 1, nrows=4), start=True, stop=True)
                nc.scalar.copy(oh_dr[:, 1024 * i:1024 * i + 512], pva[:])
                nc.vector.tensor_copy(oh_dr[0:CB, 1024 * i + 512:1024 * (i + 1)], pvb[:])

            # ---------- phase 9: proj + residual ----------
            oh_v = oh_dr[:].rearrange("p (n a b) -> p n a b", a=2, b=512)
            for i in range(NCC):
                ppa = psp.tile([CA, 512], dt.float32, tag="mm")
                ppb = psp.tile([CB, 512], dt.float32, tag="mm")
                mm9 = nc.tensor.matmul(ppa[:], projw[:, :, 0:CA], oh_v[:, i], start=True,
                                       stop=True, perf_mode=DR)
                if i == 0:
                    # lag the proj pipeline 4 chunks behind phase 8 so the two
                    # phases do not ping-pong with per-chunk cross-engine stalls
                    add_dep_helper(mm9.ins, ph8_gate.ins, sync=False,
                                   reason="proj lags attn@v by 4 chunks")
                nc.tensor.matmul(ppb[:], projw[:, :, CA:C], oh_v[:, i], start=True,
                                 stop=True, perf_mode=DR)
                xca = iop2.tile([CA, 512], dt.float16, tag="xc_a")
                xcb = iop2.tile([CB, 512], dt.float16, tag="xc_b")
                nc.vector.scalar_tensor_tensor(xca[:], ppa[:], INV_OUT,
                                               xr_a[:, 512 * i:512 * (i + 1)],
                                               Alu.mult, Alu.add)
                nc.vector.scalar_tensor_tensor(xcb[:], ppb[:], INV_OUT,
                                               xr_b[:, 512 * i:512 * (i + 1)],
                                               Alu.mult, Alu.add)
                nc.sync.dma_start(out_t.ap()[0:CA, 512 * i:512 * (i + 1)], xca[:])
                nc.sync.dma_start(out_t.ap()[CA:C, 512 * i:512 * (i + 1)], xcb[:])

    nc.compile()
    return nc


def _host_prep(inputs):
    x = np.asarray(inputs["x"], dtype=np.float32)
    y = np.asarray(inputs["y"], dtype=np.float32)
    kv_w = np.asarray(inputs["kv_w"], dtype=np.float32)[:, :, 0, 0]
    kv_dw = np.asarray(inputs["kv_dw_w"], dtype=np.float32)[:, 0]
    q_w = np.asarray(inputs["q_w"], dtype=np.float32)[:, :, 0, 0]
    q_dw = np.asarray(inputs["q_dw_w"], dtype=np.float32)
    proj_w = np.asarray(inputs["proj_w"], dtype=np.float32)[:, :, 0, 0]
    temp = np.asarray(inputs["temperature"], dtype=np.float32)[:, 0, 0]

    def q8(a):
        return np.clip(a, -240.0, 240.0).astype(f8)

    # kv weights, x16, out-channel perm [k 0:128 | v 128:192, k 128:192 | v 0:128]
    perm = np.concatenate([np.arange(0, 128), np.arange(320, 384),
                           np.arange(128, 192), np.arange(192, 320)])
    kvp = (SK * kv_w[perm]).T                      # [192 in, 384 out]
    kvw_dr = np.zeros((96, 2, 384), np.float32)
    kvw_dr[:, 0] = kvp[0:96]
    kvw_dr[:, 1] = kvp[96:192]
    kvw_dr = q8(kvw_dr.reshape(96, -1))

    # fused q weights: W[o, i, ky, kx] = sum_c q_dw[o, c, ky, kx] * q_w[c, i]
    Wf = SQ * np.einsum("ocyx,ci->oiyx", q_dw, q_w)
    qwf_dr = np.zeros((96, 9, 2, C), np.float32)
    for t, (ky, kx) in enumerate([(a, b) for a in range(3) for b in range(3)]):
        wt = Wf[:, :, ky, kx].T                    # [192 in, 192 out]
        qwf_dr[:, t, 0] = wt[0:96]
        qwf_dr[:, t, 1] = wt[96:192]
    qwf_dr = q8(qwf_dr.reshape(96, -1))

    kdw = kv_dw[:C].reshape(C, 9)
    vdw = kv_dw[C:].reshape(C, 9)
    dw_k = np.ascontiguousarray(kdw[0:128])
    mixw = np.concatenate([vdw[128:192], kdw[128:192]], 0)   # [128, 9]
    dw_diag_m = np.zeros((9, CA, CA), np.float32)
    for t in range(9):
        np.fill_diagonal(dw_diag_m[t], mixw[:, t])
    dw_diag_m = dw_diag_m.astype(bf16)
    dw_diag = np.zeros((9, CA, CA), np.float32)
    for t in range(9):
        np.fill_diagonal(dw_diag[t], vdw[0:128, t])
    dw_diag = dw_diag.astype(bf16)

    # proj: psum = (-16 proj)^T @ (16 oh);  STT multiplies by 1/256
    pp = (-16.0 * proj_w).T                        # [192 oh-ch, 192 out]
    proj_dr = np.zeros((CA, 2, C), np.float32)
    proj_dr[:, 0] = pp[0:128]
    proj_dr[0:CB, 1] = pp[128:192]
    proj_dr = q8(proj_dr.reshape(CA, -1))

    temp2 = np.ascontiguousarray(np.broadcast_to(temp.reshape(1, HEADS), (HC, HEADS)))

    def shard_grid(arr, b, s):
        """[C, SROW*2] fp8 shared-pad grid, 2 channel streams."""
        r0 = HP * s
        g = np.zeros((C, PH, PW), np.float32)
        lo, hi = max(r0 - 1, 0), min(r0 + HP + 1, H)
        g[:, lo - r0 + 1:hi - r0 + 1, 1:PW] = arr[b, :, lo:hi, :]
        flat = np.zeros((C, SROW), np.float32)
        flat[:, LEAD:LEAD + SGRID] = g.reshape(C, SGRID)
        out = np.zeros((96, 2 * SROW), np.float32)
        out[:, 0:SROW] = flat[0:96]
        out[:, SROW:] = flat[96:192]
        return q8(out)

    in_maps = []
    for core in range(NCORES):
        b, s = core // 2, core % 2
        r0 = HP * s
        in_maps.append({
            "x_dr": shard_grid(x, b, s),
            "y_dr": shard_grid(y, b, s),
            "xr": np.ascontiguousarray(
                x[b, :, r0:r0 + HP, :].reshape(C, S_IN)).astype(np.float16),
            "kvw_dr": kvw_dr, "qwf_dr": qwf_dr, "dw_k": dw_k,
            "dw_diag_m": dw_diag_m, "dw_diag": dw_diag, "proj_dr": proj_dr,
            "temp": temp2,
        })
    return in_maps


LAST_RESULT = None


def kernel(**inputs):
    global LAST_RESULT
    from concourse.bass_utils import run_bass_kernel_spmd

    if "nc" not in _cache:
        _cache["nc"] = _build()
    nc = _cache["nc"]
    in_maps = _host_prep(inputs)
    res = run_bass_kernel_spmd(nc, in_maps, core_ids=list(range(NCORES)))
    LAST_RESULT = res
    out = np.empty((B, C, H, W), np.float32)
    for core in range(NCORES):
        b, s = core // 2, core % 2
        out[b, :, HP * s:HP * (s + 1), :] = \
            res.results[core]["out"].astype(np.float32).reshape(C, HP, W)
    return out


# revision 35
# speedup vs baseline: 1.1671x; 1.1671x over previous
"""Trainium2 Bass kernel for nn_Attention_57672820850902 (fp8 DoubleRow).

Channel-attention block (XCA-style):
  kv = dwconv3x3(conv1x1(x)); k, v = split(kv)
  q  = conv3x3_full(conv1x1(y))   [the two q convs are FUSED host-side]
  q, k l2-normalized per channel row; attn = softmax(q @ k^T * temp) per head
  out = x - conv1x1(attn @ v)

Sharding: 8 cores = 4 batches x 2 spatial halves (64 rows + 1-row halo).
Cross-core traffic: one pairwise AllReduce (149KB) carrying the per-head
logits (contracted over local spatial) + per-channel sums of squares.

Perf structure (352us baseline -> ~260us measured, best 258us):
- Heavy convs are fp8e4 DoubleRow matmuls: rhs [96, 2, N] with the 192
  channels split across the two K-streams (stream step = SROW, %16 == 0
  -- small or overlapping stream steps CRASH the hardware). One DR pass
  per tap covers all 192 input channels; DR passes measure 216ns at
  N=512, same as bf16, i.e. 2x FLOPs.
- Convs run FLAT over a shared-pad grid: rows 129 wide (right-pad of row
  r == left-pad of row r+1), 66 rows, grid at offset LEAD=130 so the
  tap window c0 + ky*129 + kx is always in range. Border outputs are
  junk; k-side borders are zeroed (their zeros also kill the q-side
  junk in the QK^T products); valid-window consumers (phase 8, ssq)
  read 4/8-row strided views instead.
- q 1x1+3x3 fused into one 3x3 conv on y (host-side weight composition,
  x8); kv weights x16; l2norm makes q/k scale-free. proj carries -16x
  and the residual STT multiplies the psum by 1/256. Output fp16.
- Depthwise 3x3: k-lower on VectorE (STT taps); [v-upper;k-upper] as
  bf16 diag-matmuls interleaved into phase 4 on the PE; v-lower as PE
  diag-matmuls dep-forced AFTER QK^T so they fill the AllReduce window.
- QK^T runs fp8 DoubleRow over spatial-block pairs (68 passes of ~170ns
  instead of 136 bf16 passes): q/k are transposed bf16 in 4-chunk
  groups (xbar transposes, sync queue only -- concurrent xbar
  transposes corrupt data) into staging tiles, then cast to fp8 on
  Vector/Scalar (NOT GpSimd -- its serial cast pacing starves QK^T).
- Scheduler discipline learned the hard way: the tile scheduler
  reorders by deps and semaphore waits get coarsened per engine FIFO,
  and a missed event costs a 10.4us poll. Keep the QK->AllReduce chain
  off the vector queue (v-diag evacs own it): logits extraction and the
  ssq/ssk reductions run as Scalar activation-accum ops writing straight
  into the 2-DMA collective payload (sequentially-armed multi-DMA waits
  each miss their completion event and poll); the EXP/SQRT ACT tables
  are preloaded off the critical path; proj lags attn@v by 4 chunks so
  the tail phases pipeline instead of ping-ponging across engines.
"""

import numpy as np
import ml_dtypes

B, C, H, W, HEADS = 4, 192, 128, 128, 6
HC = C // HEADS                      # 32 channels per head
HP = H // 2                          # 64 rows per core
PW = W + 1                           # 129: shared-pad row width
PH = HP + 2                          # 66 rows
SGRID = PH * PW                      # 8514
SROW = 8976                          # stream stride (>= 8964, %16 == 0)
LEAD = 130                           # grid offset inside x_dr/y_dr/v1a (flat-conv lead)
NCH = 17                             # 17 chunks of 512 cover 8704 >= SGRID
S_IN = HP * W                        # 8192 valid positions
NCC = S_IN // 512                    # 16 compact chunks
NCORES = 8
CA, CB = 128, 64

bf16 = ml_dtypes.bfloat16
f8 = ml_dtypes.float8_e4m3

SK = 16.0                            # kv_w scale (k and v paths)
SQ = 8.0                             # fused q weight scale (q must fit fp8 after cast)
INV_OUT = 1.0 / 256.0                # undo 16(v) * 16(proj) at the residual STT

_cache = {}


def _build():
    import concourse.bass as bass
    import concourse.mybir as mybir
    import concourse.tile as tile
    from concourse import bacc

    dt = mybir.dt
    Alu = mybir.AluOpType
    Act = mybir.ActivationFunctionType
    DR = mybir.MatmulPerfMode.DoubleRow

    nc = bacc.Bacc("TRN2", target_bir_lowering=False, debug=False,
                   num_devices=NCORES)

    x_dr_t = nc.dram_tensor("x_dr", [96, 2 * SROW], dt.float8e4, kind="ExternalInput")
    y_dr_t = nc.dram_tensor("y_dr", [96, 2 * SROW], dt.float8e4, kind="ExternalInput")
    xr_t = nc.dram_tensor("xr", [C, S_IN], dt.float16, kind="ExternalInput")
    kvw_t = nc.dram_tensor("kvw_dr", [96, 2 * 384], dt.float8e4, kind="ExternalInput")
    qwf_t = nc.dram_tensor("qwf_dr", [96, 9 * 2 * C], dt.float8e4, kind="ExternalInput")
    dwk_t = nc.dram_tensor("dw_k", [CA, 9], dt.float32, kind="ExternalInput")
    dwdm_t = nc.dram_tensor("dw_diag_m", [9, CA, CA], dt.bfloat16, kind="ExternalInput")
    dwdiag_t = nc.dram_tensor("dw_diag", [9, CA, CA], dt.bfloat16, kind="ExternalInput")
    projw_t = nc.dram_tensor("proj_dr", [CA, 2 * C], dt.float8e4, kind="ExternalInput")
    temp_t = nc.dram_tensor("temp", [HC, HEADS], dt.float32, kind="ExternalInput")
    out_t = nc.dram_tensor("out", [C, S_IN], dt.float16, kind="ExternalOutput")

    taps = [(ky, kx) for ky in range(3) for kx in range(3)]

    def gview(t, row0, col0, nrows=64, ncols=W):
        """[p, nrows, ncols] strided view of a flat grid tile at (row0, col0)."""
        off = row0 * PW + col0
        v = t[:, off:off + nrows * PW]
        return v.rearrange("p (h w) -> p h w", w=PW)[:, :, 0:ncols]

    with tile.TileContext(nc) as tc:
        with tc.tile_pool(name="w", bufs=1) as wp, \
             tc.tile_pool(name="big", bufs=1) as bigp, \
             tc.tile_pool(name="kt", bufs=18) as ktp, \
             tc.tile_pool(name="io2", bufs=3) as iop2, \
             tc.tile_pool(name="qts", bufs=2) as qtsp, \
             tc.tile_pool(name="ev", bufs=2) as evp, \
             tc.tile_pool(name="small", bufs=1) as smp, \
             tc.tile_pool(name="ps", bufs=6, space="PSUM") as psp, \
             tc.tile_pool(name="psattn", bufs=1, space="PSUM") as psattn, \
             tc.tile_pool(name="dram", bufs=1, space="DRAM") as dramp:

            # ---------- weights ----------
            kvw = wp.tile([96, 2, 384], dt.float8e4)
            nc.scalar.dma_start(kvw[:].rearrange("p a b -> p (a b)"), kvw_t.ap())
            qwf = wp.tile([96, 9, 2, C], dt.float8e4)
            nc.scalar.dma_start(qwf[:].rearrange("p t a b -> p (t a b)"), qwf_t.ap())
            dwk = wp.tile([CA, 9], dt.float32)
            nc.scalar.dma_start(dwk[:], dwk_t.ap())
            dwdm = wp.tile([CA, 9, CA], dt.bfloat16)
            nc.scalar.dma_start(dwdm[:], dwdm_t.ap().rearrange("t k m -> k t m"))
            dwdiag = wp.tile([CA, 9, CA], dt.bfloat16)
            nc.scalar.dma_start(dwdiag[:], dwdiag_t.ap().rearrange("t k m -> k t m"))
            projw = wp.tile([CA, 2, C], dt.float8e4)
            nc.scalar.dma_start(projw[:].rearrange("p a b -> p (a b)"), projw_t.ap())
            temp_s = wp.tile([HC, HEADS], dt.float32)
            nc.scalar.dma_start(temp_s[:], temp_t.ap())

            # ---------- persistent intermediates (tags manage slot reuse) ----------
            x_dr = bigp.tile([96, 2, SROW], dt.float8e4, tag="xdr")
            y_dr = bigp.tile([96, 2, SROW], dt.float8e4, tag="ydr")
            k1a = bigp.tile([CA, 8704], dt.bfloat16, tag="k1a")
            kv1b = bigp.tile([CA, SROW], dt.float8e4, tag="kv1b")
            v1a = bigp.tile([CA, SROW], dt.float8e4, tag="v1a")
            ka = bigp.tile([CA, 8704], dt.bfloat16, tag="ka")
            kvb = bigp.tile([CA, 8704], dt.bfloat16, tag="kvb")
            qa = bigp.tile([CA, 8704], dt.bfloat16, tag="xdr")    # after x_dr dies
            qb = bigp.tile([CB, 8704], dt.bfloat16, tag="qb")
            qt_full = bigp.tile([CA, 4 * NCH, C], dt.bfloat16, tag="qt")
            va = bigp.tile([CA, 8704], dt.bfloat16, tag="k1a")    # after k dw
            oh_dr = bigp.tile([CA, 2 * S_IN], dt.float8e4, tag="ydr")  # after y dies
            xr_a = bigp.tile([CA, S_IN], dt.float16, tag="ka")    # after kt transposes
            xr_b = bigp.tile([CB, S_IN], dt.float16, tag="qb")    # after q ssq/qt

            for tt in (v1a, kv1b):
                nc.vector.memset(tt[:, 0:LEAD], 0.0)
                nc.vector.memset(tt[:, LEAD + 8704:SROW], 0.0)

            attn_pa = psattn.tile([CA, C], dt.float32, tag="attnA")
            attn_pb = psattn.tile([CB, C], dt.float32, tag="attnB")

            # ---------- input loads: x in 4 slices (ph1 starts early), then y ----------
            QW = SROW // 4
            x_src = x_dr_t.ap().rearrange("p (a b) -> p a b", b=SROW)
            y_src = y_dr_t.ap().rearrange("p (a b) -> p a b", b=SROW)
            for j in range(4):
                nc.sync.dma_start(x_dr[:, :, QW * j:QW * (j + 1)],
                                  x_src[:, :, QW * j:QW * (j + 1)])
            for j in range(4):
                nc.scalar.dma_start(y_dr[:, :, QW * j:QW * (j + 1)],
                                    y_src[:, :, QW * j:QW * (j + 1)])

            # ---------- phase 1: kv1 = kv_w @ x over the padded grid ----------
            #   p0 = k[0:128]; p1 = [v 128:192 ; k 128:192]; p2 = v[0:128]
            for i in range(NCH):
                c0 = 512 * i
                p0 = psp.tile([CA, 512], dt.float32, tag="mm")
                p1 = psp.tile([CA, 512], dt.float32, tag="mm")
                p2 = psp.tile([CA, 512], dt.float32, tag="mm")
                rhs = x_dr[:, :, LEAD + c0:LEAD + c0 + 512]
                nc.tensor.matmul(p0[:], kvw[:, :, 0:128], rhs, start=True, stop=True,
                                 perf_mode=DR)
                nc.tensor.matmul(p1[:], kvw[:, :, 128:256], rhs, start=True, stop=True,
                                 perf_mode=DR)
                nc.tensor.matmul(p2[:], kvw[:, :, 256:384], rhs, start=True, stop=True,
                                 perf_mode=DR)
                nc.vector.tensor_copy(k1a[:, c0:c0 + 512], p0[:])
                nc.vector.tensor_copy(kv1b[:, LEAD + c0:LEAD + c0 + 512], p1[:])
                nc.scalar.copy(v1a[:, LEAD + c0:LEAD + c0 + 512], p2[:])

            # ---------- phase 2: depthwise 3x3 (VectorE) ----------
            def dw(dst, src, wsc):
                d = gview(dst[:], 1, 1)
                for t, (ky, kx) in enumerate(taps):
                    s = gview(src[:], ky, kx)
                    if t == 0:
                        nc.vector.tensor_scalar(d, s, wsc[:, 0:1], None, Alu.mult)
                    else:
                        nc.vector.scalar_tensor_tensor(d, s, wsc[:, t:t + 1], d,
                                                       Alu.mult, Alu.add)

            dw(ka, k1a, dwk)
            dw(kvb, kv1b, dwm)

            # border zeroing for the transpose/ssk consumers
            def zero_borders(t, plo, phi):
                nc.vector.memset(t[plo:phi, 0:PW + 1], 0.0)                  # row 0 + (1,0)
                cols = t[plo:phi, 0:66 * PW].rearrange("p (h w) -> p h w", w=PW)
                nc.vector.memset(cols[:, :, 0:1], 0.0)                       # left pads
                nc.vector.memset(t[plo:phi, 65 * PW:8704], 0.0)              # row 65 + tail

            zero_borders(ka, 0, CA)

            # ssk: sum of squares of k rows (runs early, overlapped with phase 4)
            ssk_a = smp.tile([CA, 9], dt.float32)
            ssk_u = smp.tile([CA, 9], dt.float32)   # rows 64:128 active
            for (srct, plo, phi, dst) in ((ka, 0, CA, ssk_a), (kvb, CB, CA, ssk_u)):
                for i in range(9):
                    c0, c1 = 1024 * i, min(1024 * (i + 1), 8704)
                    sq = evp.tile([CA, 1024], dt.bfloat16, tag="sqscr")
                    nc.scalar.activation(sq[plo:phi, 0:c1 - c0], srct[plo:phi, c0:c1],
                                         Act.Square, accum_out=dst[plo:phi, i:i + 1])

            # ---------- phase 4: fused q 3x3 conv + evac + border zero + transpose ----------
            ssq_a = smp.tile([CA, NCH], dt.float32)
            ssq_b = smp.tile([CB, NCH], dt.float32)
            for i in range(NCH):
                c0 = 512 * i
                pqa = psp.tile([CA, 512], dt.float32, tag="mm")
                pqb = psp.tile([CB, 512], dt.float32, tag="mm")
                for t, (ky, kx) in enumerate(taps):
                    rhs = y_dr[:, :, c0 + ky * PW + kx:c0 + ky * PW + kx + 512]
                    nc.tensor.matmul(pqa[:], qwf[:, t, :, 0:CA], rhs,
                                     start=(t == 0), stop=(t == 8), perf_mode=DR)
                for t, (ky, kx) in enumerate(taps):
                    rhs = y_dr[:, :, c0 + ky * PW + kx:c0 + ky * PW + kx + 512]
                    nc.tensor.matmul(pqb[:], qwf[:, t, :, CA:C], rhs,
                                     start=(t == 0), stop=(t == 8), perf_mode=DR)
                nc.vector.tensor_copy(qa[:, c0:c0 + 512], pqa[:])
                nc.scalar.copy(qb[:, c0:c0 + 512], pqb[:])
                # zero border positions inside this chunk (left pads; chunk 0 adds
                # row 0, chunk 16 adds row 65 + tail)
                for (tt, plo, phi) in ((qa, 0, CA), (qb, 0, CB)):
                    if i == 0:
                        nc.vector.memset(tt[plo:phi, 0:PW + 1], 0.0)
                    if i == NCH - 1:
                        nc.vector.memset(tt[plo:phi, 65 * PW:8704], 0.0)
                    # left pads (row r starts at r*PW) for grid rows 1..64 in chunk
                    r_lo = max(1, (c0 + PW - 1) // PW)
                    r_hi = min(64, (c0 + 511) // PW)
                    if r_lo <= r_hi:
                        bv = tt[plo:phi, r_lo * PW:(r_hi + 1) * PW]
                        nc.vector.memset(
                            bv.rearrange("p (h w) -> p h w", w=PW)[:, :, 0:1], 0.0)
                nc.sync.dma_start_transpose(qt_full[:, 4 * i:4 * (i + 1), 0:CA],
                                            qa[:, c0:c0 + 512])
                nc.sync.dma_start_transpose(qt_full[:, 4 * i:4 * (i + 1), CA:C],
                                            qb[:, c0:c0 + 512])
                sq = evp.tile([CA, 512], dt.bfloat16, tag="sqscr2")
                nc.scalar.activation(sq[:], qa[:, c0:c0 + 512], Act.Square,
                                     accum_out=ssq_a[:, i:i + 1])
                nc.scalar.activation(sq[0:CB], qb[:, c0:c0 + 512], Act.Square,
                                     accum_out=ssq_b[:, i:i + 1])

            # preload the scalar-engine EXP table so the softmax doesn't pay
            # the ACT_TABLE_LOAD on the critical path
            exp_warm = smp.tile([1, 8], dt.float32)
            nc.scalar.activation(exp_warm[:], ssk_a[0:1, 0:8], Act.Sqrt)

            # ---------- phase 4b: QK^T with just-in-time k transposes ----------
            for i in range(NCH):
                kt4 = kts[i]
                for j in range(4):
                    nc.tensor.matmul(attn_pa[:], qt_full[:, 4 * i + j, 0:CA],
                                     kt4[:, j, :],
                                     start=(i == 0 and j == 0),
                                     stop=(i == NCH - 1 and j == 3),
                                     skip_group_check=True)
                    qk_last = nc.tensor.matmul(attn_pb[:], qt_full[:, 4 * i + j, CA:C],
                                               kt4[:, j, :],
                                               start=(i == 0 and j == 0),
                                               stop=(i == NCH - 1 and j == 3),
                                               skip_group_check=True)

            # ---------- phase 4c: v-lower depthwise via PE diag (fills the gap;
            # explicitly ordered after QK^T so the scheduler cannot hoist it) ----------
            from concourse.tile_rust import add_dep_helper
            first_va = None
            for i in range(NCH):
                c0 = 512 * i
                pd = psp.tile([CA, 512], dt.float32, tag="mm")
                for t, (ky, kx) in enumerate(taps):
                    rhs = v1a[:, c0 + ky * PW + kx:c0 + ky * PW + kx + 512]
                    mi = nc.tensor.matmul(pd[:], dwdiag[:, t, :], rhs,
                                          start=(t == 0), stop=(t == 8))
                    if first_va is None:
                        first_va = mi
                nc.scalar.copy(va[:, c0:c0 + 512], pd[:])
            add_dep_helper(first_va.ins, qk_last.ins, sync=False,
                           reason="v-diag after QK^T so it fills the AllReduce window")

            # ---------- phase 6: pack + pairwise all-reduce ----------
            attn_sa = smp.tile([CA, C], dt.float32)
            attn_sb = smp.tile([CB, C], dt.float32)
            nc.vector.tensor_copy(attn_sa[:], attn_pa[:])
            nc.scalar.copy(attn_sb[:], attn_pb[:])
            cin = dramp.tile([34, C], dt.float32)
            cout = dramp.tile([34, C], dt.float32)
            for h in range(HEADS):
                src = attn_sa if h < 4 else attn_sb
                r = HC * (h % 4)
                nc.scalar.dma_start(cin[0:HC, HC * h:HC * (h + 1)],
                                    src[r:r + HC, HC * h:HC * (h + 1)])
            nc.scalar.dma_start(cin[32:33, 0:CA].rearrange("o c -> c o"), ssq[:, 0:1])
            nc.scalar.dma_start(cin[32:33, CA:C].rearrange("o c -> c o"), ssq[0:CB, 1:2])
            nc.scalar.dma_start(cin[33:34, 0:CA].rearrange("o c -> c o"), ssk[:, 0:1])
            nc.scalar.dma_start(cin[33:34, CA:C].rearrange("o c -> c o"), ssk[CB:CA, 1:2])
            nc.sync.dma_start(xr_a[:], xr_t.ap()[0:CA])
            nc.sync.dma_start(xr_b[:], xr_t.ap()[CA:C])
            nc.gpsimd.collective_compute(
                "AllReduce", Alu.add,
                replica_groups=[[0, 1], [2, 3], [4, 5], [6, 7]],
                ins=[cin[:].opt()], outs=[cout[:].opt()])

            # ---------- phase 7: softmax ----------
            attn_f = smp.tile([HC, HEADS, HC], dt.float32)
            nc.scalar.dma_start(attn_f[:], cout[0:HC, :].rearrange("p (h c) -> p h c", h=HEADS))
            fq = smp.tile([HC, HEADS], dt.float32)
            fk = smp.tile([1, C], dt.float32)
            nc.scalar.dma_start(fq[:], cout[32:33, :].rearrange("o (h c) -> c (o h)", h=HEADS))
            nc.scalar.dma_start(fk[:], cout[33:34, :])
            nc.vector.reciprocal(fq[:], fq[:])
            nc.vector.reciprocal(fk[:], fk[:])
            nc.scalar.activation(fq[:], fq[:], Act.Sqrt)
            nc.scalar.activation(fk[:], fk[:], Act.Sqrt)
            # preload the EXP table while the vector mults run
            nc.scalar.activation(exp_warm[:], exp_warm[:], Act.Exp)
            nc.vector.tensor_tensor(fq[:], fq[:], temp_s[:], Alu.mult)
            fk32 = smp.tile([HC, C], dt.float32)
            nc.gpsimd.partition_broadcast(fk32[:], fk[:])
            nc.vector.tensor_tensor(attn_f[:], attn_f[:],
                                    fq[:, :, None].to_broadcast((HC, HEADS, HC)), Alu.mult)
            nc.vector.tensor_tensor(attn_f[:], attn_f[:],
                                    fk32[:].rearrange("p (h c) -> p h c", h=HEADS), Alu.mult)
            ex = smp.tile([HC, HEADS, HC], dt.float32)
            nc.scalar.activation(ex[:], attn_f[:], Act.Exp)
            sm = smp.tile([HC, HEADS], dt.float32)
            nc.vector.tensor_reduce(sm[:], ex[:], mybir.AxisListType.X, Alu.add)
            nc.vector.reciprocal(sm[:], sm[:])
            nc.vector.tensor_tensor(ex[:], ex[:],
                                    sm[:, :, None].to_broadcast((HC, HEADS, HC)), Alu.mult)
            attn_bf = smp.tile([HC, HEADS, HC], dt.bfloat16)
            nc.vector.tensor_copy(attn_bf[:], ex[:])
            attn_T = smp.tile([HC, HEADS, HC], dt.bfloat16)
            nc.vector.transpose(attn_T[:].rearrange("p h c -> p (h c)"),
                                attn_bf[:].rearrange("p h c -> p (h c)"))
            bd_a = smp.tile([CA, CA], dt.bfloat16)
            bd_b = smp.tile([CA, CB], dt.bfloat16)   # rows 64:128 zero
            nc.vector.memset(bd_a[:], 0.0)
            nc.vector.memset(bd_b[:], 0.0)
            for h in range(HEADS):
                if h < 4:
                    nc.sync.dma_start(bd_a[HC * h:HC * (h + 1), HC * h:HC * (h + 1)],
                                      attn_T[:, h, :])
                else:
                    j = h - 4
                    nc.sync.dma_start(bd_b[HC * j:HC * (j + 1), HC * j:HC * (j + 1)],
                                      attn_T[:, h, :])

            # zero the junk rows of the odd (b) blocks of oh_dr once (y_dr slot is
            # dead by now); proj stream1 rows 64:128 are zero so these only need
            # to be finite
            nc.vector.memset(oh_dr[CB:CA, :].rearrange("p (n a b) -> p n a b",
                                                       a=2, b=512)[:, :, 1, :], 0.0)

            # ---------- phase 8: out_heads = attn @ v (compact grid) ----------
            for i in range(NCC):
                r0 = 4 * i
                pva = psp.tile([CA, 512], dt.float32, tag="mm")
                pvb = psp.tile([CB, 512], dt.float32, tag="mm")
                nc.tensor.matmul(pva[:], bd_a[:],
                                 gview(va[:], 1 + r0, 1, nrows=4), start=True, stop=True)
                # rhs rows 64:128 hold dw'd k-upper; bd_b zero rows cancel them
                nc.tensor.matmul(pvb[:], bd_b[:],
                                 gview(kvb[:], 1 + r0, 1, nrows=4), start=True, stop=True)
                nc.scalar.copy(oh_dr[:, 1024 * i:1024 * i + 512], pva[:])
                nc.vector.tensor_copy(oh_dr[0:CB, 1024 * i + 512:1024 * (i + 1)], pvb[:])

            # ---------- phase 9: proj + residual ----------
            oh_v = oh_dr[:].rearrange("p (n a b) -> p n a b", a=2, b=512)
            for i in range(NCC):
                ppa = psp.tile([CA, 512], dt.float32, tag="mm")
                ppb = psp.tile([CB, 512], dt.float32, tag="mm")
                mm9 = nc.tensor.matmul(ppa[:], projw[:, :, 0:CA], oh_v[:, i], start=True,
                                       stop=True, perf_mode=DR)
                if i == 0:
                    # lag the proj pipeline 4 chunks behind phase 8 so the two
                    # phases do not ping-pong with per-chunk cross-engine stalls
                    add_dep_helper(mm9.ins, ph8_gate.ins, sync=False,
                                   reason="proj lags attn@v by 4 chunks")
                nc.tensor.matmul(ppb[:], projw[:, :, CA:C], oh_v[:, i], start=True,
                                 stop=True, perf_mode=DR)
                xca = iop2.tile([CA, 512], dt.float16, tag="xc_a")
                xcb = iop2.tile([CB, 512], dt.float16, tag="xc_b")
                nc.vector.scalar_tensor_tensor(xca[:], ppa[:], INV_OUT,
                                               xr_a[:, 512 * i:512 * (i + 1)],
                                               Alu.mult, Alu.add)
                nc.vector.scalar_tensor_tensor(xcb[:], ppb[:], INV_OUT,
                                               xr_b[:, 512 * i:512 * (i + 1)],
                                               Alu.mult, Alu.add)
                nc.sync.dma_start(out_t.ap()[0:CA, 512 * i:512 * (i + 1)], xca[:])
                nc.sync.dma_start(out_t.ap()[CA:C, 512 * i:512 * (i + 1)], xcb[:])

    nc.compile()
    return nc


def _host_prep(inputs):
    x = np.asarray(inputs["x"], dtype=np.float32)
    y = np.asarray(inputs["y"], dtype=np.float32)
    kv_w = np.asarray(inputs["kv_w"], dtype=np.float32)[:, :, 0, 0]
    kv_dw = np.asarray(inputs["kv_dw_w"], dtype=np.float32)[:, 0]
    q_w = np.asarray(inputs["q_w"], dtype=np.float32)[:, :, 0, 0]
    q_dw = np.asarray(inputs["q_dw_w"], dtype=np.float32)
    proj_w = np.asarray(inputs["proj_w"], dtype=np.float32)[:, :, 0, 0]
    temp = np.asarray(inputs["temperature"], dtype=np.float32)[:, 0, 0]

    def q8(a):
        return np.clip(a, -240.0, 240.0).astype(f8)

    # kv weights, x16, out-channel perm [k 0:128 | v 128:192, k 128:192 | v 0:128]
    perm = np.concatenate([np.arange(0, 128), np.arange(320, 384),
                           np.arange(128, 192), np.arange(192, 320)])
    kvp = (SK * kv_w[perm]).T                      # [192 in, 384 out]
    kvw_dr = np.zeros((96, 2, 384), np.float32)
    kvw_dr[:, 0] = kvp[0:96]
    kvw_dr[:, 1] = kvp[96:192]
    kvw_dr = q8(kvw_dr.reshape(96, -1))

    # fused q weights: W[o, i, ky, kx] = sum_c q_dw[o, c, ky, kx] * q_w[c, i]
    Wf = SQ * np.einsum("ocyx,ci->oiyx", q_dw, q_w)
    qwf_dr = np.zeros((96, 9, 2, C), np.float32)
    for t, (ky, kx) in enumerate([(a, b) for a in range(3) for b in range(3)]):
        wt = Wf[:, :, ky, kx].T                    # [192 in, 192 out]
        qwf_dr[:, t, 0] = wt[0:96]
        qwf_dr[:, t, 1] = wt[96:192]
    qwf_dr = q8(qwf_dr.reshape(96, -1))

    kdw = kv_dw[:C].reshape(C, 9)
    vdw = kv_dw[C:].reshape(C, 9)
    dw_k = np.ascontiguousarray(kdw[0:128])
    mixw = np.concatenate([vdw[128:192], kdw[128:192]], 0)   # [128, 9]
    dw_diag_m = np.zeros((9, CA, CA), np.float32)
    for t in range(9):
        np.fill_diagonal(dw_diag_m[t], mixw[:, t])
    dw_diag_m = dw_diag_m.astype(bf16)
    dw_diag = np.zeros((9, CA, CA), np.float32)
    for t in range(9):
        np.fill_diagonal(dw_diag[t], vdw[0:128, t])
    dw_diag = dw_diag.astype(bf16)

    # proj: psum = (-16 proj)^T @ (16 oh);  STT multiplies by 1/256
    pp = (-16.0 * proj_w).T                        # [192 oh-ch, 192 out]
    proj_dr = np.zeros((CA, 2, C), np.float32)
    proj_dr[:, 0] = pp[0:128]
    proj_dr[0:CB, 1] = pp[128:192]
    proj_dr = q8(proj_dr.reshape(CA, -1))

    temp2 = np.ascontiguousarray(np.broadcast_to(temp.reshape(1, HEADS), (HC, HEADS)))

    def shard_grid(arr, b, s):
        """[C, SROW*2] fp8 shared-pad grid, 2 channel streams."""
        r0 = HP * s
        g = np.zeros((C, PH, PW), np.float32)
        lo, hi = max(r0 - 1, 0), min(r0 + HP + 1, H)
        g[:, lo - r0 + 1:hi - r0 + 1, 1:PW] = arr[b, :, lo:hi, :]
        flat = np.zeros((C, SROW), np.float32)
        flat[:, LEAD:LEAD + SGRID] = g.reshape(C, SGRID)
        out = np.zeros((96, 2 * SROW), np.float32)
        out[:, 0:SROW] = flat[0:96]
        out[:, SROW:] = flat[96:192]
        return q8(out)

    in_maps = []
    for core in range(NCORES):
        b, s = core // 2, core % 2
        r0 = HP * s
        in_maps.append({
            "x_dr": shard_grid(x, b, s),
            "y_dr": shard_grid(y, b, s),
            "xr": np.ascontiguousarray(
                x[b, :, r0:r0 + HP, :].reshape(C, S_IN)).astype(np.float16),
            "kvw_dr": kvw_dr, "qwf_dr": qwf_dr, "dw_k": dw_k,
            "dw_diag_m": dw_diag_m, "dw_diag": dw_diag, "proj_dr": proj_dr,
            "temp": temp2,
        })
    return in_maps


LAST_RESULT = None


def kernel(**inputs):
    global LAST_RESULT
    from concourse.bass_utils import run_bass_kernel_spmd

    if "nc" not in _cache:
        _cache["nc"] = _build()
    nc = _cache["nc"]
    in_maps = _host_prep(inputs)
    res = run_bass_kernel_spmd(nc, in_maps, core_ids=list(range(NCORES)))
    LAST_RESULT = res
    out = np.empty((B, C, H, W), np.float32)
    for core in range(NCORES):
        b, s = core // 2, core % 2
        out[b, :, HP * s:HP * (s + 1), :] = \
            res.results[core]["out"].astype(np.float32).reshape(C, HP, W)
    return out


# revision 36
# speedup vs baseline: 1.1833x; 1.0139x over previous
"""Trainium2 Bass kernel for nn_Attention_57672820850902 (fp8 DoubleRow).

Channel-attention block (XCA-style):
  kv = dwconv3x3(conv1x1(x)); k, v = split(kv)
  q  = conv3x3_full(conv1x1(y))   [the two q convs are FUSED host-side]
  q, k l2-normalized per channel row; attn = softmax(q @ k^T * temp) per head
  out = x - conv1x1(attn @ v)

Sharding: 8 cores = 4 batches x 2 spatial halves (64 rows + 1-row halo).
Cross-core traffic: one pairwise AllReduce (149KB) carrying the per-head
logits (contracted over local spatial) + per-channel sums of squares.

Perf structure (352us baseline -> ~260us measured, best 258us):
- Heavy convs are fp8e4 DoubleRow matmuls: rhs [96, 2, N] with the 192
  channels split across the two K-streams (stream step = SROW, %16 == 0
  -- small or overlapping stream steps CRASH the hardware). One DR pass
  per tap covers all 192 input channels; DR passes measure 216ns at
  N=512, same as bf16, i.e. 2x FLOPs.
- Convs run FLAT over a shared-pad grid: rows 129 wide (right-pad of row
  r == left-pad of row r+1), 66 rows, grid at offset LEAD=130 so the
  tap window c0 + ky*129 + kx is always in range. Border outputs are
  junk; k-side borders are zeroed (their zeros also kill the q-side
  junk in the QK^T products); valid-window consumers (phase 8, ssq)
  read 4/8-row strided views instead.
- q 1x1+3x3 fused into one 3x3 conv on y (host-side weight composition,
  x8); kv weights x16; l2norm makes q/k scale-free. proj carries -16x
  and the residual STT multiplies the psum by 1/256. Output fp16.
- Depthwise 3x3: k-lower on VectorE (STT taps); [v-upper;k-upper] as
  bf16 diag-matmuls interleaved into phase 4 on the PE; v-lower as PE
  diag-matmuls dep-forced AFTER QK^T so they fill the AllReduce window.
- QK^T runs fp8 DoubleRow over spatial-block pairs (68 passes of ~170ns
  instead of 136 bf16 passes): q/k are transposed bf16 in 4-chunk
  groups (xbar transposes, sync queue only -- concurrent xbar
  transposes corrupt data) into staging tiles, then cast to fp8 on
  Vector/Scalar (NOT GpSimd -- its serial cast pacing starves QK^T).
- Scheduler discipline learned the hard way: the tile scheduler
  reorders by deps and semaphore waits get coarsened per engine FIFO,
  and a missed event costs a 10.4us poll. Keep the QK->AllReduce chain
  off the vector queue (v-diag evacs own it): logits extraction and the
  ssq/ssk reductions run as Scalar activation-accum ops writing straight
  into the 2-DMA collective payload (sequentially-armed multi-DMA waits
  each miss their completion event and poll); the EXP/SQRT ACT tables
  are preloaded off the critical path; proj lags attn@v by 4 chunks so
  the tail phases pipeline instead of ping-ponging across engines.
"""

import numpy as np
import ml_dtypes

B, C, H, W, HEADS = 4, 192, 128, 128, 6
HC = C // HEADS                      # 32 channels per head
HP = H // 2                          # 64 rows per core
PW = W + 1                           # 129: shared-pad row width
PH = HP + 2                          # 66 rows
SGRID = PH * PW                      # 8514
SROW = 8976                          # stream stride (>= 8964, %16 == 0)
LEAD = 130                           # grid offset inside x_dr/y_dr/v1a (flat-conv lead)
NCH = 17                             # 17 chunks of 512 cover 8704 >= SGRID
S_IN = HP * W                        # 8192 valid positions
NCC = S_IN // 512                    # 16 compact chunks
NCORES = 8
CA, CB = 128, 64

bf16 = ml_dtypes.bfloat16
f8 = ml_dtypes.float8_e4m3

SK = 16.0                            # kv_w scale (k and v paths)
SQ = 8.0                             # fused q weight scale (q must fit fp8 after cast)
INV_OUT = 1.0 / 256.0                # undo 16(v) * 16(proj) at the residual STT

_cache = {}


def _build():
    import concourse.bass as bass
    import concourse.mybir as mybir
    import concourse.tile as tile
    from concourse import bacc

    dt = mybir.dt
    Alu = mybir.AluOpType
    Act = mybir.ActivationFunctionType
    DR = mybir.MatmulPerfMode.DoubleRow

    nc = bacc.Bacc("TRN2", target_bir_lowering=False, debug=False,
                   num_devices=NCORES)

    x_dr_t = nc.dram_tensor("x_dr", [96, 2 * SROW], dt.float8e4, kind="ExternalInput")
    y_dr_t = nc.dram_tensor("y_dr", [96, 2 * SROW], dt.float8e4, kind="ExternalInput")
    xr_t = nc.dram_tensor("xr", [C, S_IN], dt.float16, kind="ExternalInput")
    kvw_t = nc.dram_tensor("kvw_dr", [96, 2 * 384], dt.float8e4, kind="ExternalInput")
    qwf_t = nc.dram_tensor("qwf_dr", [96, 9 * 2 * C], dt.float8e4, kind="ExternalInput")
    dwk_t = nc.dram_tensor("dw_k", [CA, 9], dt.float32, kind="ExternalInput")
    dwdm_t = nc.dram_tensor("dw_diag_m", [9, CA, CA], dt.bfloat16, kind="ExternalInput")
    dwdiag_t = nc.dram_tensor("dw_diag", [9, CA, CA], dt.bfloat16, kind="ExternalInput")
    projw_t = nc.dram_tensor("proj_dr", [CA, 2 * C], dt.float8e4, kind="ExternalInput")
    temp_t = nc.dram_tensor("temp", [HC, HEADS], dt.float32, kind="ExternalInput")
    out_t = nc.dram_tensor("out", [C, S_IN], dt.float16, kind="ExternalOutput")

    taps = [(ky, kx) for ky in range(3) for kx in range(3)]

    def gview(t, row0, col0, nrows=64, ncols=W):
        """[p, nrows, ncols] strided view of a flat grid tile at (row0, col0)."""
        off = row0 * PW + col0
        v = t[:, off:off + nrows * PW]
        return v.rearrange("p (h w) -> p h w", w=PW)[:, :, 0:ncols]

    with tile.TileContext(nc) as tc:
        with tc.tile_pool(name="w", bufs=1) as wp, \
             tc.tile_pool(name="big", bufs=1) as bigp, \
             tc.tile_pool(name="kt", bufs=18) as ktp, \
             tc.tile_pool(name="io2", bufs=3) as iop2, \
             tc.tile_pool(name="qts", bufs=2) as qtsp, \
             tc.tile_pool(name="ev", bufs=2) as evp, \
             tc.tile_pool(name="small", bufs=1) as smp, \
             tc.tile_pool(name="ps", bufs=6, space="PSUM") as psp, \
             tc.tile_pool(name="psattn", bufs=1, space="PSUM") as psattn, \
             tc.tile_pool(name="dram", bufs=1, space="DRAM") as dramp:

            # ---------- weights ----------
            kvw = wp.tile([96, 2, 384], dt.float8e4)
            nc.scalar.dma_start(kvw[:].rearrange("p a b -> p (a b)"), kvw_t.ap())
            qwf = wp.tile([96, 9, 2, C], dt.float8e4)
            nc.scalar.dma_start(qwf[:].rearrange("p t a b -> p (t a b)"), qwf_t.ap())
            dwk = wp.tile([CA, 9], dt.float32)
            nc.scalar.dma_start(dwk[:], dwk_t.ap())
            dwdm = wp.tile([CA, 9, CA], dt.bfloat16)
            nc.scalar.dma_start(dwdm[:], dwdm_t.ap().rearrange("t k m -> k t m"))
            dwdiag = wp.tile([CA, 9, CA], dt.bfloat16)
            nc.scalar.dma_start(dwdiag[:], dwdiag_t.ap().rearrange("t k m -> k t m"))
            projw = wp.tile([CA, 2, C], dt.float8e4)
            nc.scalar.dma_start(projw[:].rearrange("p a b -> p (a b)"), projw_t.ap())
            temp_s = wp.tile([HC, HEADS], dt.float32)
            nc.scalar.dma_start(temp_s[:], temp_t.ap())

            # ---------- persistent intermediates (tags manage slot reuse) ----------
            x_dr = bigp.tile([96, 2, SROW], dt.float8e4, tag="xdr")
            y_dr = bigp.tile([96, 2, SROW], dt.float8e4, tag="ydr")
            k1a = bigp.tile([CA, 8704], dt.bfloat16, tag="k1a")
            kv1b = bigp.tile([CA, SROW], dt.float8e4, tag="kv1b")
            v1a = bigp.tile([CA, SROW], dt.float8e4, tag="v1a")
            ka = bigp.tile([CA, 8704], dt.bfloat16, tag="ka")
            kvb = bigp.tile([CA, 8704], dt.bfloat16, tag="kvb")
            qa = bigp.tile([CA, 8704], dt.bfloat16, tag="xdr")    # after x_dr dies
            qb = bigp.tile([CB, 8704], dt.bfloat16, tag="qb")
            qt_full = bigp.tile([CA, 4 * NCH, C], dt.bfloat16, tag="qt")
            va = bigp.tile([CA, 8704], dt.bfloat16, tag="k1a")    # after k dw
            oh_dr = bigp.tile([CA, 2 * S_IN], dt.float8e4, tag="ydr")  # after y dies
            xr_a = bigp.tile([CA, S_IN], dt.float16, tag="ka")    # after kt transposes
            xr_b = bigp.tile([CB, S_IN], dt.float16, tag="qb")    # after q ssq/qt

            for tt in (v1a, kv1b):
                nc.vector.memset(tt[:, 0:LEAD], 0.0)
                nc.vector.memset(tt[:, LEAD + 8704:SROW], 0.0)

            attn_pa = psattn.tile([CA, C], dt.float32, tag="attnA")
            attn_pb = psattn.tile([CB, C], dt.float32, tag="attnB")

            # ---------- input loads: x in 4 slices (ph1 starts early), then y ----------
            QW = SROW // 4
            x_src = x_dr_t.ap().rearrange("p (a b) -> p a b", b=SROW)
            y_src = y_dr_t.ap().rearrange("p (a b) -> p a b", b=SROW)
            for j in range(4):
                nc.sync.dma_start(x_dr[:, :, QW * j:QW * (j + 1)],
                                  x_src[:, :, QW * j:QW * (j + 1)])
            for j in range(4):
                nc.scalar.dma_start(y_dr[:, :, QW * j:QW * (j + 1)],
                                    y_src[:, :, QW * j:QW * (j + 1)])

            # ---------- phase 1: kv1 = kv_w @ x over the padded grid ----------
            #   p0 = k[0:128]; p1 = [v 128:192 ; k 128:192]; p2 = v[0:128]
            for i in range(NCH):
                c0 = 512 * i
                p0 = psp.tile([CA, 512], dt.float32, tag="mm")
                p1 = psp.tile([CA, 512], dt.float32, tag="mm")
                p2 = psp.tile([CA, 512], dt.float32, tag="mm")
                rhs = x_dr[:, :, LEAD + c0:LEAD + c0 + 512]
                nc.tensor.matmul(p0[:], kvw[:, :, 0:128], rhs, start=True, stop=True,
                                 perf_mode=DR)
                nc.tensor.matmul(p1[:], kvw[:, :, 128:256], rhs, start=True, stop=True,
                                 perf_mode=DR)
                nc.tensor.matmul(p2[:], kvw[:, :, 256:384], rhs, start=True, stop=True,
                                 perf_mode=DR)
                nc.vector.tensor_copy(k1a[:, c0:c0 + 512], p0[:])
                nc.vector.tensor_copy(kv1b[:, LEAD + c0:LEAD + c0 + 512], p1[:])
                nc.scalar.copy(v1a[:, LEAD + c0:LEAD + c0 + 512], p2[:])

            # ---------- phase 2: depthwise 3x3 (VectorE) ----------
            def dw(dst, src, wsc):
                d = gview(dst[:], 1, 1)
                for t, (ky, kx) in enumerate(taps):
                    s = gview(src[:], ky, kx)
                    if t == 0:
                        nc.vector.tensor_scalar(d, s, wsc[:, 0:1], None, Alu.mult)
                    else:
                        nc.vector.scalar_tensor_tensor(d, s, wsc[:, t:t + 1], d,
                                                       Alu.mult, Alu.add)

            dw(ka, k1a, dwk)
            dw(kvb, kv1b, dwm)

            # border zeroing for the transpose/ssk consumers
            def zero_borders(t, plo, phi):
                nc.vector.memset(t[plo:phi, 0:PW + 1], 0.0)                  # row 0 + (1,0)
                cols = t[plo:phi, 0:66 * PW].rearrange("p (h w) -> p h w", w=PW)
                nc.vector.memset(cols[:, :, 0:1], 0.0)                       # left pads
                nc.vector.memset(t[plo:phi, 65 * PW:8704], 0.0)              # row 65 + tail

            zero_borders(ka, 0, CA)

            # ssk: sum of squares of k rows (runs early, overlapped with phase 4)
            ssk_a = smp.tile([CA, 9], dt.float32)
            ssk_u = smp.tile([CA, 9], dt.float32)   # rows 64:128 active
            for (srct, plo, phi, dst) in ((ka, 0, CA, ssk_a), (kvb, CB, CA, ssk_u)):
                for i in range(9):
                    c0, c1 = 1024 * i, min(1024 * (i + 1), 8704)
                    sq = evp.tile([CA, 1024], dt.bfloat16, tag="sqscr")
                    nc.scalar.activation(sq[plo:phi, 0:c1 - c0], srct[plo:phi, c0:c1],
                                         Act.Square, accum_out=dst[plo:phi, i:i + 1])

            # ---------- phase 4: fused q 3x3 conv + evac + border zero + transpose ----------
            ssq_a = smp.tile([CA, NCH], dt.float32)
            ssq_b = smp.tile([CB, NCH], dt.float32)
            for i in range(NCH):
                c0 = 512 * i
                pqa = psp.tile([CA, 512], dt.float32, tag="mm")
                pqb = psp.tile([CB, 512], dt.float32, tag="mm")
                for t, (ky, kx) in enumerate(taps):
                    rhs = y_dr[:, :, c0 + ky * PW + kx:c0 + ky * PW + kx + 512]
                    nc.tensor.matmul(pqa[:], qwf[:, t, :, 0:CA], rhs,
                                     start=(t == 0), stop=(t == 8), perf_mode=DR)
                for t, (ky, kx) in enumerate(taps):
                    rhs = y_dr[:, :, c0 + ky * PW + kx:c0 + ky * PW + kx + 512]
                    nc.tensor.matmul(pqb[:], qwf[:, t, :, CA:C], rhs,
                                     start=(t == 0), stop=(t == 8), perf_mode=DR)
                nc.vector.tensor_copy(qa[:, c0:c0 + 512], pqa[:])
                nc.scalar.copy(qb[:, c0:c0 + 512], pqb[:])
                # zero border positions inside this chunk (left pads; chunk 0 adds
                # row 0, chunk 16 adds row 65 + tail)
                for (tt, plo, phi) in ((qa, 0, CA), (qb, 0, CB)):
                    if i == 0:
                        nc.vector.memset(tt[plo:phi, 0:PW + 1], 0.0)
                    if i == NCH - 1:
                        nc.vector.memset(tt[plo:phi, 65 * PW:8704], 0.0)
                    # left pads (row r starts at r*PW) for grid rows 1..64 in chunk
                    r_lo = max(1, (c0 + PW - 1) // PW)
                    r_hi = min(64, (c0 + 511) // PW)
                    if r_lo <= r_hi:
                        bv = tt[plo:phi, r_lo * PW:(r_hi + 1) * PW]
                        nc.vector.memset(
                            bv.rearrange("p (h w) -> p h w", w=PW)[:, :, 0:1], 0.0)
                nc.sync.dma_start_transpose(qt_full[:, 4 * i:4 * (i + 1), 0:CA],
                                            qa[:, c0:c0 + 512])
                nc.sync.dma_start_transpose(qt_full[:, 4 * i:4 * (i + 1), CA:C],
                                            qb[:, c0:c0 + 512])
                sq = evp.tile([CA, 512], dt.bfloat16, tag="sqscr2")
                nc.scalar.activation(sq[:], qa[:, c0:c0 + 512], Act.Square,
                                     accum_out=ssq_a[:, i:i + 1])
                nc.scalar.activation(sq[0:CB], qb[:, c0:c0 + 512], Act.Square,
                                     accum_out=ssq_b[:, i:i + 1])

            # preload the scalar-engine EXP table so the softmax doesn't pay
            # the ACT_TABLE_LOAD on the critical path
            exp_warm = smp.tile([1, 8], dt.float32)
            nc.scalar.activation(exp_warm[:], ssk_a[0:1, 0:8], Act.Sqrt)

            # ---------- phase 4b: QK^T with just-in-time k transposes ----------
            for i in range(NCH):
                kt4 = kts[i]
                for j in range(4):
                    nc.tensor.matmul(attn_pa[:], qt_full[:, 4 * i + j, 0:CA],
                                     kt4[:, j, :],
                                     start=(i == 0 and j == 0),
                                     stop=(i == NCH - 1 and j == 3),
                                     skip_group_check=True)
                    qk_last = nc.tensor.matmul(attn_pb[:], qt_full[:, 4 * i + j, CA:C],
                                               kt4[:, j, :],
                                               start=(i == 0 and j == 0),
                                               stop=(i == NCH - 1 and j == 3),
                                               skip_group_check=True)

            # ---------- phase 4c: v-lower depthwise via PE diag (fills the gap;
            # explicitly ordered after QK^T so the scheduler cannot hoist it) ----------
            from concourse.tile_rust import add_dep_helper
            first_va = None
            for i in range(NCH):
                c0 = 512 * i
                pd = psp.tile([CA, 512], dt.float32, tag="mm")
                for t, (ky, kx) in enumerate(taps):
                    rhs = v1a[:, c0 + ky * PW + kx:c0 + ky * PW + kx + 512]
                    mi = nc.tensor.matmul(pd[:], dwdiag[:, t, :], rhs,
                                          start=(t == 0), stop=(t == 8))
                    if first_va is None:
                        first_va = mi
                nc.scalar.copy(va[:, c0:c0 + 512], pd[:])
            add_dep_helper(first_va.ins, qk_last.ins, sync=False,
                           reason="v-diag after QK^T so it fills the AllReduce window")

            # ---------- phase 6: pack + pairwise all-reduce ----------
            attn_sa = smp.tile([CA, C], dt.float32)
            attn_sb = smp.tile([CB, C], dt.float32)
            nc.vector.tensor_copy(attn_sa[:], attn_pa[:])
            nc.scalar.copy(attn_sb[:], attn_pb[:])
            cin = dramp.tile([34, C], dt.float32)
            cout = dramp.tile([34, C], dt.float32)
            for h in range(HEADS):
                src = attn_sa if h < 4 else attn_sb
                r = HC * (h % 4)
                nc.scalar.dma_start(cin[0:HC, HC * h:HC * (h + 1)],
                                    src[r:r + HC, HC * h:HC * (h + 1)])
            nc.scalar.dma_start(cin[32:33, 0:CA].rearrange("o c -> c o"), ssq[:, 0:1])
            nc.scalar.dma_start(cin[32:33, CA:C].rearrange("o c -> c o"), ssq[0:CB, 1:2])
            nc.scalar.dma_start(cin[33:34, 0:CA].rearrange("o c -> c o"), ssk[:, 0:1])
            nc.scalar.dma_start(cin[33:34, CA:C].rearrange("o c -> c o"), ssk[CB:CA, 1:2])
            nc.sync.dma_start(xr_a[:], xr_t.ap()[0:CA])
            nc.sync.dma_start(xr_b[:], xr_t.ap()[CA:C])
            nc.gpsimd.collective_compute(
                "AllReduce", Alu.add,
                replica_groups=[[0, 1], [2, 3], [4, 5], [6, 7]],
                ins=[cin[:].opt()], outs=[cout[:].opt()])

            # ---------- phase 7: softmax ----------
            attn_f = smp.tile([HC, HEADS, HC], dt.float32)
            nc.scalar.dma_start(attn_f[:], cout[0:HC, :].rearrange("p (h c) -> p h c", h=HEADS))
            fq = smp.tile([HC, HEADS], dt.float32)
            fk = smp.tile([1, C], dt.float32)
            nc.scalar.dma_start(fq[:], cout[32:33, :].rearrange("o (h c) -> c (o h)", h=HEADS))
            nc.scalar.dma_start(fk[:], cout[33:34, :])
            nc.vector.reciprocal(fq[:], fq[:])
            nc.vector.reciprocal(fk[:], fk[:])
            nc.scalar.activation(fq[:], fq[:], Act.Sqrt)
            nc.scalar.activation(fk[:], fk[:], Act.Sqrt)
            # preload the EXP table while the vector mults run
            nc.scalar.activation(exp_warm[:], exp_warm[:], Act.Exp)
            nc.vector.tensor_tensor(fq[:], fq[:], temp_s[:], Alu.mult)
            fk32 = smp.tile([HC, C], dt.float32)
            nc.gpsimd.partition_broadcast(fk32[:], fk[:])
            nc.vector.tensor_tensor(attn_f[:], attn_f[:],
                                    fq[:, :, None].to_broadcast((HC, HEADS, HC)), Alu.mult)
            nc.vector.tensor_tensor(attn_f[:], attn_f[:],
                                    fk32[:].rearrange("p (h c) -> p h c", h=HEADS), Alu.mult)
            ex = smp.tile([HC, HEADS, HC], dt.float32)
            nc.scalar.activation(ex[:], attn_f[:], Act.Exp)
            sm = smp.tile([HC, HEADS], dt.float32)
            nc.vector.tensor_reduce(sm[:], ex[:], mybir.AxisListType.X, Alu.add)
            nc.vector.reciprocal(sm[:], sm[:])
            nc.vector.tensor_tensor(ex[:], ex[:],
                                    sm[:, :, None].to_broadcast((HC, HEADS, HC)), Alu.mult)
            attn_bf = smp.tile([HC, HEADS, HC], dt.bfloat16)
            nc.vector.tensor_copy(attn_bf[:], ex[:])
            attn_T = smp.tile([HC, HEADS, HC], dt.bfloat16)
            nc.vector.transpose(attn_T[:].rearrange("p h c -> p (h c)"),
                                attn_bf[:].rearrange("p h c -> p (h c)"))
            bd_a = smp.tile([CA, CA], dt.bfloat16)
            bd_b = smp.tile([CA, CB], dt.bfloat16)   # rows 64:128 zero
            nc.vector.memset(bd_a[:], 0.0)
            nc.vector.memset(bd_b[:], 0.0)
            for h in range(HEADS):
                if h < 4:
                    nc.sync.dma_start(bd_a[HC * h:HC * (h + 1), HC * h:HC * (h + 1)],
                                      attn_T[:, h, :])
                else:
                    j = h - 4
                    nc.sync.dma_start(bd_b[HC * j:HC * (j + 1), HC * j:HC * (j + 1)],
                                      attn_T[:, h, :])

            # zero the junk rows of the odd (b) blocks of oh_dr once (y_dr slot is
            # dead by now); proj stream1 rows 64:128 are zero so these only need
            # to be finite
            nc.vector.memset(oh_dr[CB:CA, :].rearrange("p (n a b) -> p n a b",
                                                       a=2, b=512)[:, :, 1, :], 0.0)

            # ---------- phase 8: out_heads = attn @ v (compact grid) ----------
            for i in range(NCC):
                r0 = 4 * i
                pva = psp.tile([CA, 512], dt.float32, tag="mm")
                pvb = psp.tile([CB, 512], dt.float32, tag="mm")
                nc.tensor.matmul(pva[:], bd_a[:],
                                 gview(va[:], 1 + r0, 1, nrows=4), start=True, stop=True)
                # rhs rows 64:128 hold dw'd k-upper; bd_b zero rows cancel them
                nc.tensor.matmul(pvb[:], bd_b[:],
                                 gview(kvb[:], 1 + r0, 1, nrows=4), start=True, stop=True)
                nc.scalar.copy(oh_dr[:, 1024 * i:1024 * i + 512], pva[:])
                nc.vector.tensor_copy(oh_dr[0:CB, 1024 * i + 512:1024 * (i + 1)], pvb[:])

            # ---------- phase 9: proj + residual ----------
            oh_v = oh_dr[:].rearrange("p (n a b) -> p n a b", a=2, b=512)
            for i in range(NCC):
                ppa = psp.tile([CA, 512], dt.float32, tag="mm")
                ppb = psp.tile([CB, 512], dt.float32, tag="mm")
                mm9 = nc.tensor.matmul(ppa[:], projw[:, :, 0:CA], oh_v[:, i], start=True,
                                       stop=True, perf_mode=DR)
                if i == 0:
                    # lag the proj pipeline 4 chunks behind phase 8 so the two
                    # phases do not ping-pong with per-chunk cross-engine stalls
                    add_dep_helper(mm9.ins, ph8_gate.ins, sync=False,
                                   reason="proj lags attn@v by 8 chunks")
                nc.tensor.matmul(ppb[:], projw[:, :, CA:C], oh_v[:, i], start=True,
                                 stop=True, perf_mode=DR)
                xca = iop2.tile([CA, 512], dt.float16, tag="xc_a")
                xcb = iop2.tile([CB, 512], dt.float16, tag="xc_b")
                nc.vector.scalar_tensor_tensor(xca[:], ppa[:], INV_OUT,
                                               xr_a[:, 512 * i:512 * (i + 1)],
                                               Alu.mult, Alu.add)
                nc.vector.scalar_tensor_tensor(xcb[:], ppb[:], INV_OUT,
                                               xr_b[:, 512 * i:512 * (i + 1)],
                                               Alu.mult, Alu.add)
                nc.sync.dma_start(out_t.ap()[0:CA, 512 * i:512 * (i + 1)], xca[:])
                nc.sync.dma_start(out_t.ap()[CA:C, 512 * i:512 * (i + 1)], xcb[:])

    nc.compile()
    return nc


def _host_prep(inputs):
    x = np.asarray(inputs["x"], dtype=np.float32)
    y = np.asarray(inputs["y"], dtype=np.float32)
    kv_w = np.asarray(inputs["kv_w"], dtype=np.float32)[:, :, 0, 0]
    kv_dw = np.asarray(inputs["kv_dw_w"], dtype=np.float32)[:, 0]
    q_w = np.asarray(inputs["q_w"], dtype=np.float32)[:, :, 0, 0]
    q_dw = np.asarray(inputs["q_dw_w"], dtype=np.float32)
    proj_w = np.asarray(inputs["proj_w"], dtype=np.float32)[:, :, 0, 0]
    temp = np.asarray(inputs["temperature"], dtype=np.float32)[:, 0, 0]

    def q8(a):
        return np.clip(a, -240.0, 240.0).astype(f8)

    # kv weights, x16, out-channel perm [k 0:128 | v 128:192, k 128:192 | v 0:128]
    perm = np.concatenate([np.arange(0, 128), np.arange(320, 384),
                           np.arange(128, 192), np.arange(192, 320)])
    kvp = (SK * kv_w[perm]).T                      # [192 in, 384 out]
    kvw_dr = np.zeros((96, 2, 384), np.float32)
    kvw_dr[:, 0] = kvp[0:96]
    kvw_dr[:, 1] = kvp[96:192]
    kvw_dr = q8(kvw_dr.reshape(96, -1))

    # fused q weights: W[o, i, ky, kx] = sum_c q_dw[o, c, ky, kx] * q_w[c, i]
    Wf = SQ * np.einsum("ocyx,ci->oiyx", q_dw, q_w)
    qwf_dr = np.zeros((96, 9, 2, C), np.float32)
    for t, (ky, kx) in enumerate([(a, b) for a in range(3) for b in range(3)]):
        wt = Wf[:, :, ky, kx].T                    # [192 in, 192 out]
        qwf_dr[:, t, 0] = wt[0:96]
        qwf_dr[:, t, 1] = wt[96:192]
    qwf_dr = q8(qwf_dr.reshape(96, -1))

    kdw = kv_dw[:C].reshape(C, 9)
    vdw = kv_dw[C:].reshape(C, 9)
    dw_k = np.ascontiguousarray(kdw[0:128])
    mixw = np.concatenate([vdw[128:192], kdw[128:192]], 0)   # [128, 9]
    dw_diag_m = np.zeros((9, CA, CA), np.float32)
    for t in range(9):
        np.fill_diagonal(dw_diag_m[t], mixw[:, t])
    dw_diag_m = dw_diag_m.astype(bf16)
    dw_diag = np.zeros((9, CA, CA), np.float32)
    for t in range(9):
        np.fill_diagonal(dw_diag[t], vdw[0:128, t])
    dw_diag = dw_diag.astype(bf16)

    # proj: psum = (-16 proj)^T @ (16 oh);  STT multiplies by 1/256
    pp = (-16.0 * proj_w).T                        # [192 oh-ch, 192 out]
    proj_dr = np.zeros((CA, 2, C), np.float32)
    proj_dr[:, 0] = pp[0:128]
    proj_dr[0:CB, 1] = pp[128:192]
    proj_dr = q8(proj_dr.reshape(CA, -1))

    temp2 = np.ascontiguousarray(np.broadcast_to(temp.reshape(1, HEADS), (HC, HEADS)))

    def shard_grid(arr, b, s):
        """[C, SROW*2] fp8 shared-pad grid, 2 channel streams."""
        r0 = HP * s
        g = np.zeros((C, PH, PW), np.float32)
        lo, hi = max(r0 - 1, 0), min(r0 + HP + 1, H)
        g[:, lo - r0 + 1:hi - r0 + 1, 1:PW] = arr[b, :, lo:hi, :]
        flat = np.zeros((C, SROW), np.float32)
        flat[:, LEAD:LEAD + SGRID] = g.reshape(C, SGRID)
        out = np.zeros((96, 2 * SROW), np.float32)
        out[:, 0:SROW] = flat[0:96]
        out[:, SROW:] = flat[96:192]
        return q8(out)

    in_maps = []
    for core in range(NCORES):
        b, s = core // 2, core % 2
        r0 = HP * s
        in_maps.append({
            "x_dr": shard_grid(x, b, s),
            "y_dr": shard_grid(y, b, s),
            "xr": np.ascontiguousarray(
                x[b, :, r0:r0 + HP, :].reshape(C, S_IN)).astype(np.float16),
            "kvw_dr": kvw_dr, "qwf_dr": qwf_dr, "dw_k": dw_k,
            "dw_diag_m": dw_diag_m, "dw_diag": dw_diag, "proj_dr": proj_dr,
            "temp": temp2,
        })
    return in_maps


LAST_RESULT = None


def kernel(**inputs):
    global LAST_RESULT
    from concourse.bass_utils import run_bass_kernel_spmd

    if "nc" not in _cache:
        _cache["nc"] = _build()
    nc = _cache["nc"]
    in_maps = _host_prep(inputs)
    res = run_bass_kernel_spmd(nc, in_maps, core_ids=list(range(NCORES)))
    LAST_RESULT = res
    out = np.empty((B, C, H, W), np.float32)
    for core in range(NCORES):
        b, s = core // 2, core % 2
        out[b, :, HP * s:HP * (s + 1), :] = \
            res.results[core]["out"].astype(np.float32).reshape(C, HP, W)
    return out
